# revision 1
# baseline (speedup 1.0000x reference)
"""Trainium2 Bass kernel for CachedMultiheadAttention (sliding-window + ALiBi).

Sharding: 8 cores = 2 batches x 4 head-quartets. Core c handles batch c//4 and
heads [4*(c%4), 4*(c%4)+4). Each core computes QKV projection for its heads,
banded attention (causal + 512 window + ALiBi), and a partial out-projection
over its heads' 256 embedding columns. Host sums the 4 partials per batch.

On-chip pipeline (per core):
  x^T (f32r) --PE--> Q^T,K^T,V^T (bf16, [2heads*64, T] stacked)
  V^T --transpose-DMA--> V natural [t,64] + ones column (rowsum trick)
  S^T[j,q] = K^T.T @ Q^T (bf16 PE), + precomputed bias tile (band -inf + ALiBi,
  max-pre-shifted so no online softmax), exp on ACT -> P^T bf16
  AO^T[d,q] (+rowsum row) = V_aug.T @ P^T (PE, M=65, accumulate 5 j-blocks)
  normalize via reciprocal + gpsimd partition_broadcast; out_proj in f32r.
"""
import math

import numpy as np
import ml_dtypes

import concourse.bass as bass
import concourse.tile as tile
from concourse.masks import make_identity
from concourse import bacc, mybir
from concourse.bass_utils import run_bass_kernel_spmd

F32 = mybir.dt.float32
F32R = mybir.dt.float32r
BF16 = mybir.dt.bfloat16

B, T, E, H, HD, W = 2, 2048, 1024, 16, 64, 512
NCORES = 8
HL = 4                # local heads per core
NT = T // 128         # 16 t-blocks
NEG = -60000.0        # masked-logit bias (exp -> 0, no inf arithmetic)

_CACHE = {}


def _get_slopes(n):
    def p2(m):
        start = 2 ** (-(2 ** (-(math.log2(m) - 3))))
        return [start * start**i for i in range(m)]
    if math.log2(n) % 1 == 0:
        return p2(n)
    c = 2 ** math.floor(math.log2(n))
    return p2(c) + _get_slopes(2 * c)[0::2][: n - c]


def _build(dbg=False):
    nc = bacc.Bacc("TRN2", target_bir_lowering=False, debug=False, num_devices=NCORES)
    xT = nc.dram_tensor("xT", [8, 128, T], F32R, kind="ExternalInput").ap()
    wqkv = nc.dram_tensor("wqkv", [8, 128, 768], F32R, kind="ExternalInput").ap()
    wo = nc.dram_tensor("wo", [2, 128, E], F32R, kind="ExternalInput").ap()
    biasd = nc.dram_tensor("biasd", [HL, 128, 640], BF16, kind="ExternalInput").ap()
    outT = nc.dram_tensor("outT", [8, 128, T], BF16, kind="ExternalOutput").ap()
    if dbg:
        d_qkvT = nc.dram_tensor("d_qkvT", [128, 6, T], BF16, kind="ExternalOutput").ap()
        d_vnat = nc.dram_tensor("d_vnat", [128, HL, NT, HD + 1], BF16, kind="ExternalOutput").ap()
        d_pth = nc.dram_tensor("d_pth", [128, NT, 640], BF16, kind="ExternalOutput").ap()
        d_ao2T = nc.dram_tensor("d_ao2T", [128, 2, T], F32R, kind="ExternalOutput").ap()

    with tile.TileContext(nc) as tc:
        with (
            tc.tile_pool(name="singles", bufs=1) as singles,
            tc.tile_pool(name="xp", bufs=2) as xp,
            tc.tile_pool(name="ptp", bufs=3) as ptp,
            tc.tile_pool(name="sprep", bufs=3) as sprep,
            tc.tile_pool(name="smallp", bufs=3) as smallp,
            tc.tile_pool(name="evp", bufs=3) as evp,
            tc.tile_pool(name="mm", bufs=3, space="PSUM") as mmp,
            tc.tile_pool(name="s1", bufs=3, space="PSUM") as s1p,
            tc.tile_pool(name="aop", bufs=2, space="PSUM") as aop,
        ):
            # --- one-time loads (per-chunk so the first matmuls start early;
            # wqkv on the ACT queue, x on SP, the rest on gpsimd) ---
            wqkv_sb = singles.tile([128, 8, 768], F32R)
            for ec in range(8):
                nc.gpsimd.dma_start(
                    wqkv_sb[:, ec, :], wqkv.rearrange("c p m -> p c m")[:, ec, :])
            wo_sb = singles.tile([128, 2, E], F32R)
            nc.gpsimd.dma_start(wo_sb[:], wo.rearrange("c p f -> p c f"))
            bias_sb = singles.tile([128, HL, 640], BF16)
            nc.gpsimd.dma_start(bias_sb[:], biasd.rearrange("h p c -> p h c"))

            qkvT = singles.tile([128, 6, T], BF16)   # slots: Qp0 Qp1 Kp0 Kp1 Vp0 Vp1
            vnat = singles.tile([128, HL, NT, HD + 1], BF16)
            nc.gpsimd.memset(vnat[:], 1.0)           # ones column survives at [...,64]
            ao2T = singles.tile([128, 2, T], F32R)   # normalized AO^T, [2h*64, pair, t]

            # --- phase 1: QKV projection (f32r, N=512) ---
            for tb in range(4):
                xc = xp.tile([128, 8, 512], F32R)
                for ec in range(8):
                    eng = nc.sync if ec % 2 == 0 else nc.scalar
                    eng.dma_start(
                        xc[:, ec, :],
                        xT[ec, :, tb * 512:(tb + 1) * 512],
                    )
                for m in range(6):
                    pt = mmp.tile([128, 512], F32, tag="mm512")
                    for ec in range(8):
                        nc.tensor.matmul(
                            pt[:],
                            lhsT=wqkv_sb[:, ec, m * 128:(m + 1) * 128],
                            rhs=xc[:, ec, :],
                            start=(ec == 0), stop=(ec == 7),
                        )
                    nc.vector.tensor_copy(qkvT[:, m, tb * 512:(tb + 1) * 512], pt[:])

            # --- phase 1.5: V natural layout via PE transpose (bf16) ---
            ident = singles.tile([128, 64], BF16)
            make_identity(nc, ident[0:64, :])
            make_identity(nc, ident[64:128, :])
            for h in range(HL):
                r0, sq = (h % 2) * 64, h // 2
                for jb in range(NT):
                    vtr = s1p.tile([128, HD], BF16, tag="s128")
                    nc.tensor.transpose(
                        vtr[:],
                        qkvT[r0:r0 + 64, 4 + sq, jb * 128:(jb + 1) * 128],
                        ident[r0:r0 + 64, :],
                    )
                    nc.vector.tensor_copy(vnat[:, h, jb, 0:HD], vtr[:])

            # --- phase 2: attention, head-pair interleaved ---
            # Even head lives on PE row-strips 0-1 (base partition 0), odd head
            # on strips 2-3 (base 64): their K=64 S-matmuls run concurrently,
            # and each head's exp/mul latency hides under the other's matmuls.
            for sq in range(2):
                ptha = ptp.tile([128, NT, 640], BF16, tag="pth")
                pthb = ptp.tile([128, NT, 640], BF16, tag="pth")
                pths = [ptha, pthb]
                for jb in range(NT):
                    nq = min(5, NT - jb)
                    qw = nq * 128
                    w0 = min(qw, 512)
                    for hh in range(2):
                        h = 2 * sq + hh
                        r0 = hh * 64
                        pth = pths[hh]
                        praw = sprep.tile([128, 640], BF16, tag="praw")
                        s5 = mmp.tile([128, 512], F32, tag="mm512")
                        nc.tensor.matmul(
                            s5[:, 0:w0],
                            lhsT=qkvT[r0:r0 + 64, 2 + sq, jb * 128:(jb + 1) * 128],
                            rhs=qkvT[r0:r0 + 64, sq, jb * 128:jb * 128 + w0],
                            start=True, stop=True,
                        )
                        nc.scalar.activation(
                            out=praw[:, 0:w0], in_=s5[:, 0:w0],
                            func=mybir.ActivationFunctionType.Exp,
                        )
                        if qw > 512:
                            s1 = s1p.tile([128, 128], F32, tag="s128")
                            nc.tensor.matmul(
                                s1[:],
                                lhsT=qkvT[r0:r0 + 64, 2 + sq, jb * 128:(jb + 1) * 128],
                                rhs=qkvT[r0:r0 + 64, sq, jb * 128 + 512:jb * 128 + qw],
                                start=True, stop=True,
                            )
                            nc.scalar.activation(
                                out=praw[:, 512:qw], in_=s1[:],
                                func=mybir.ActivationFunctionType.Exp,
                            )
                        # P = exp(S) * exp(bias): all-bf16 SBUF mul (DVE fast)
                        nc.vector.tensor_tensor(
                            out=pth[:, jb, 0:qw], in0=praw[:, 0:qw],
                            in1=bias_sb[:, h, 0:qw], op=mybir.AluOpType.mult,
                        )

                if dbg and sq == 0:
                    nc.sync.dma_start(d_pth[:], pths[0][:])
                # AV batched per 4-query-block group: jb=4g covers the whole
                # 512-col group (start=True clears has_written); the other
                # jb's accumulate partial column ranges.
                for g in range(4):
                    for hh in range(2):
                        h = 2 * sq + hh
                        r0 = hh * 64
                        pth = pths[hh]
                        ao = aop.tile([65, 512], F32, tag="ao")
                        jbs = [4 * g] + [jb for jb in range(max(0, 4 * g - 4), 4 * g + 4)
                                         if jb != 4 * g]
                        for i, jb in enumerate(jbs):
                            qb_lo = max(4 * g, jb)
                            qb_hi = min(4 * g + 3, jb + 4)
                            wdt = (qb_hi - qb_lo + 1) * 128
                            ao_off = (qb_lo - 4 * g) * 128
                            p_off = (qb_lo - jb) * 128
                            nc.tensor.matmul(
                                ao[:, ao_off:ao_off + wdt],
                                lhsT=vnat[:, h, jb, :],
                                rhs=pth[:, jb, p_off:p_off + wdt],
                                start=(i == 0), stop=(i == len(jbs) - 1),
                                skip_group_check=True,
                            )
                        rec = smallp.tile([1, 512], F32, tag="rec")
                        nc.vector.reciprocal(rec[:], ao[64:65, :])
                        bc = smallp.tile([64, 512], F32, tag="bc")
                        nc.gpsimd.partition_broadcast(bc[:], rec[:])
                        nc.vector.tensor_tensor(
                            out=ao2T[r0:r0 + 64, sq, g * 512:(g + 1) * 512],
                            in0=ao[0:64, :], in1=bc[:], op=mybir.AluOpType.mult,
                        )

            if dbg:
                nc.sync.dma_start(d_qkvT[:], qkvT[:])
                nc.sync.dma_start(d_vnat[:], vnat[:])
                nc.sync.dma_start(d_ao2T[:], ao2T[:])

            # --- phase 3: out projection (f32r, partial over 256 local e-cols) ---
            for tb in range(4):
                for fc in range(8):
                    po = mmp.tile([128, 512], F32, tag="mm512")
                    for c2 in range(2):
                        nc.tensor.matmul(
                            po[:],
                            lhsT=wo_sb[:, c2, fc * 128:(fc + 1) * 128],
                            rhs=ao2T[:, c2, tb * 512:(tb + 1) * 512],
                            start=(c2 == 0), stop=(c2 == 1),
                        )
                    ev = evp.tile([128, 512], BF16, tag="ev")
                    if fc % 2 == 0:
                        nc.vector.tensor_copy(ev[:], po[:])
                    else:
                        nc.scalar.copy(ev[:], po[:])
                    nc.sync.dma_start(outT[fc, :, tb * 512:(tb + 1) * 512], ev[:])

    nc.compile()
    return nc


def _host_inputs(query, in_proj_weight, out_proj_weight):
    """Per-core input maps (numpy only)."""
    slopes = np.asarray(_get_slopes(H), np.float32)
    q32 = np.asarray(query, np.float32)
    w_in = np.asarray(in_proj_weight, np.float32)
    w_out = np.asarray(out_proj_weight, np.float32)

    # band+alibi bias tiles, shift-invariant per head: [h, jj, cc]
    jj = np.arange(128)[:, None]
    cc = np.arange(640)[None, :]
    allowed = (cc >= jj) & (cc - jj <= W)
    in_maps = []
    for c in range(NCORES):
        b, hq = divmod(c, 4)
        heads = np.arange(4 * hq, 4 * hq + HL)
        rows = (heads[:, None] * HD + np.arange(HD)[None, :]).reshape(-1)  # 256 rows
        wq = w_in[rows, :] * (1.0 / math.sqrt(HD))
        wk = w_in[E + rows, :]
        wv = w_in[2 * E + rows, :]
        w_loc = np.concatenate([wq, wk, wv], axis=0)          # [768, E]
        wqkv = np.ascontiguousarray(w_loc.T.reshape(8, 128, 768), np.float32)

        xT = np.ascontiguousarray(q32[b].T.reshape(8, 128, T), np.float32)

        wo_loc = np.ascontiguousarray(w_out[:, rows].T.reshape(2, 128, E), np.float32)

        biasd = np.empty((HL, 128, 640), ml_dtypes.bfloat16)
        for hl in range(HL):
            s = slopes[4 * hq + hl]
            eb = np.where(allowed, np.exp(-s * (cc - jj).astype(np.float64)), 0.0)
            biasd[hl] = eb.astype(ml_dtypes.bfloat16)

        in_maps.append({"xT": xT, "wqkv": wqkv, "wo": wo_loc, "biasd": biasd})
    return in_maps


def _assemble(results):
    out = np.zeros((B, T, E), np.float32)
    for c in range(NCORES):
        b = c // 4
        part = np.asarray(results[c]["outT"]).astype(np.float32)  # [8,128,T]
        out[b] += part.reshape(E, T).T
    return out


def kernel(query, in_proj_weight, out_proj_weight, num_heads, sliding_window_size):
    assert int(num_heads) == H and int(sliding_window_size) == W
    assert query.shape == (B, T, E)
    if "nc" not in _CACHE:
        _CACHE["nc"] = _build()
    in_maps = _host_inputs(query, in_proj_weight, out_proj_weight)
    res = run_bass_kernel_spmd(_CACHE["nc"], in_maps, list(range(NCORES))).results
    return _assemble(res)



# revision 11
# speedup vs baseline: 1.3723x; 1.3723x over previous
"""Trainium2 Bass kernel for CachedMultiheadAttention (sliding-window + ALiBi).

Sharding: 8 cores = 2 batches x 4 head-quartets. Core c handles batch c//4 and
heads [4*(c%4), 4*(c%4)+4). Each core computes QKV projection for its heads,
banded attention (causal + 512 window + ALiBi), and a partial out-projection
over its heads' 256 embedding columns. Host sums the 4 partials per batch.

v2 design (all bf16 matmuls, PE kept saturated):
  - Q^T/K^T produced slot-by-slot so S-matmuls start 2/5 into the projection.
  - V produced in NATURAL layout straight from the projection (no PE
    transposes), with a 64-wide ones block per head: the AV matmul output
    rows 0-63 are AO^T and rows 64-127 are the softmax denominator Z
    replicated 64x -- the partition broadcast comes free from the matmul.
  - Normalize = reciprocal_approx_fast + one multiply (no serial
    reciprocal/partition_broadcast chain).
  - exp on ACT; band+ALiBi multiplicative bias on DVE/GpSimd alternating.
  - out-proj per 512-col group interleaved with the last AV groups.
"""
import math

import numpy as np
import ml_dtypes

import concourse.bass as bass
import concourse.tile as tile
from concourse import bacc, mybir
from concourse.bass_utils import run_bass_kernel_spmd

F32 = mybir.dt.float32
BF16 = mybir.dt.bfloat16

B, T, E, H, HD, W = 2, 2048, 1024, 16, 64, 512
NCORES = 8
HL = 4                # local heads per core
NT = T // 128         # 16 t-blocks

_CACHE = {}


def _get_slopes(n):
    def p2(m):
        start = 2 ** (-(2 ** (-(math.log2(m) - 3))))
        return [start * start**i for i in range(m)]
    if math.log2(n) % 1 == 0:
        return p2(n)
    c = 2 ** math.floor(math.log2(n))
    return p2(c) + _get_slopes(2 * c)[0::2][: n - c]


def _build(dbg=False):
    nc = bacc.Bacc("TRN2", target_bir_lowering=False, debug=False, num_devices=NCORES)
    xT = nc.dram_tensor("xT", [8, 128, T], BF16, kind="ExternalInput").ap()
    wqk = nc.dram_tensor("wqk", [8, 128, 512], BF16, kind="ExternalInput").ap()
    wvT = nc.dram_tensor("wvT", [8, 128, 256], BF16, kind="ExternalInput").ap()
    wo = nc.dram_tensor("wo", [2, 128, E], BF16, kind="ExternalInput").ap()
    biasd = nc.dram_tensor("biasd", [HL, 128, 640], BF16, kind="ExternalInput").ap()
    outT = nc.dram_tensor("outT", [8, 128, T], BF16, kind="ExternalOutput").ap()
    if dbg:
        d_qkvT = nc.dram_tensor("d_qkvT", [128, 4, T], BF16, kind="ExternalOutput").ap()
        d_vnat = nc.dram_tensor("d_vnat", [128, NT, HL, 128], BF16, kind="ExternalOutput").ap()
        d_pth = nc.dram_tensor("d_pth", [128, NT, 640], BF16, kind="ExternalOutput").ap()
        d_ao2T = nc.dram_tensor("d_ao2T", [128, 2, T], BF16, kind="ExternalOutput").ap()

    with tile.TileContext(nc) as tc:
        with (
            tc.tile_pool(name="singles", bufs=1) as singles,
            tc.tile_pool(name="ptp", bufs=4) as ptp,
            tc.tile_pool(name="sprep", bufs=3) as sprep,
            tc.tile_pool(name="recp", bufs=2) as recp,
            tc.tile_pool(name="evp", bufs=3) as evp,
            tc.tile_pool(name="mm", bufs=4, space="PSUM") as mmp,
            tc.tile_pool(name="s1", bufs=2, space="PSUM") as s1p,
            tc.tile_pool(name="accp", bufs=2, space="PSUM") as accp,
        ):
            # --- one-time loads ---
            wqk_sb = singles.tile([128, 8, 512], BF16)
            for ec in range(8):
                nc.gpsimd.dma_start(
                    wqk_sb[:, ec, :], wqk.rearrange("c p m -> p c m")[:, ec, :])
            wvT_sb = singles.tile([128, 8, 256], BF16)
            nc.gpsimd.dma_start(wvT_sb[:], wvT.rearrange("c p m -> p c m"))
            wo_sb = singles.tile([128, 2, E], BF16)
            nc.gpsimd.dma_start(wo_sb[:], wo.rearrange("c p f -> p c f"))
            bias_sb = singles.tile([128, HL, 640], BF16)
            nc.gpsimd.dma_start(bias_sb[:], biasd.rearrange("h p c -> p h c"))

            xsb = singles.tile([128, 8, T], BF16)
            for th in range(2):
                for ec in range(8):
                    eng = nc.sync if ec % 2 == 0 else nc.scalar
                    eng.dma_start(
                        xsb[:, ec, th * 1024:(th + 1) * 1024],
                        xT[ec, :, th * 1024:(th + 1) * 1024])

            qkvT = singles.tile([128, 4, T], BF16)   # slots: K0 Q0 K1 Q1
            vnat = singles.tile([128, NT, HL, 128], BF16)
            nc.gpsimd.memset(vnat[:], 1.0)           # ones survive at [..., 64:128]
            ao2T = singles.tile([128, 2, T], BF16)   # normalized AO^T

            # --- emit helpers ---
            def emit_qk(slots, copy_eng):
                for s in slots:
                    for tb in range(4):
                        pt = mmp.tile([128, 512], F32, tag="mm512")
                        for ec in range(8):
                            nc.tensor.matmul(
                                pt[:],
                                lhsT=wqk_sb[:, ec, s * 128:(s + 1) * 128],
                                rhs=xsb[:, ec, tb * 512:(tb + 1) * 512],
                                start=(ec == 0), stop=(ec == 7),
                            )
                        if copy_eng is nc.scalar:
                            nc.scalar.copy(qkvT[:, s, tb * 512:(tb + 1) * 512], pt[:])
                        else:
                            copy_eng.tensor_copy(
                                qkvT[:, s, tb * 512:(tb + 1) * 512], pt[:])

            def emit_v(tb16):
                pv = accp.tile([128, 256], F32, tag="acc")
                for ec in range(8):
                    nc.tensor.matmul(
                        pv[:],
                        lhsT=xsb[:, ec, tb16 * 128:(tb16 + 1) * 128],
                        rhs=wvT_sb[:, ec, :],
                        start=(ec == 0), stop=(ec == 7),
                    )
                nc.vector.tensor_copy(
                    vnat[:, tb16, :, 0:HD],
                    pv[:].rearrange("p (h d) -> p h d", h=HL))

            def emit_s(sq, pths, jb):
                ks, qs = 2 * sq, 2 * sq + 1
                qw = min(5, NT - jb) * 128
                w0 = min(qw, 512)
                for hh in range(2):
                    h = 2 * sq + hh
                    r0 = hh * 64
                    pth = pths[hh]
                    praw = sprep.tile([128, 640], BF16, tag="praw")
                    s5 = mmp.tile([128, 512], F32, tag="mm512")
                    nc.tensor.matmul(
                        s5[:, 0:w0],
                        lhsT=qkvT[r0:r0 + 64, ks, jb * 128:(jb + 1) * 128],
                        rhs=qkvT[r0:r0 + 64, qs, jb * 128:jb * 128 + w0],
                        start=True, stop=True,
                    )
                    nc.scalar.activation(
                        out=praw[:, 0:w0], in_=s5[:, 0:w0],
                        func=mybir.ActivationFunctionType.Exp,
                    )
                    if qw > 512:
                        s1 = s1p.tile([128, 128], F32, tag="s128")
                        nc.tensor.matmul(
                            s1[:],
                            lhsT=qkvT[r0:r0 + 64, ks, jb * 128:(jb + 1) * 128],
                            rhs=qkvT[r0:r0 + 64, qs, jb * 128 + 512:jb * 128 + qw],
                            start=True, stop=True,
                        )
                        nc.scalar.activation(
                            out=praw[:, 512:qw], in_=s1[:],
                            func=mybir.ActivationFunctionType.Exp,
                        )
                    eng = nc.vector if hh == 0 else nc.gpsimd
                    eng.tensor_tensor(
                        out=pth[:, jb, 0:qw], in0=praw[:, 0:qw],
                        in1=bias_sb[:, h, 0:qw], op=mybir.AluOpType.mult,
                    )

            def emit_av_group(sq, pths, g):
                for hh in range(2):
                    h = 2 * sq + hh
                    r0 = hh * 64
                    pth = pths[hh]
                    ao = accp.tile([128, 512], F32, tag="acc")
                    jbs = [4 * g] + [jb for jb in range(max(0, 4 * g - 4), 4 * g + 4)
                                     if jb != 4 * g]
                    for i, jb in enumerate(jbs):
                        qb_lo = max(4 * g, jb)
                        qb_hi = min(4 * g + 3, jb + 4)
                        wdt = (qb_hi - qb_lo + 1) * 128
                        ao_off = (qb_lo - 4 * g) * 128
                        p_off = (qb_lo - jb) * 128
                        nc.tensor.matmul(
                            ao[:, ao_off:ao_off + wdt],
                            lhsT=vnat[:, jb, h, :],
                            rhs=pth[:, jb, p_off:p_off + wdt],
                            start=(i == 0), stop=(i == len(jbs) - 1),
                            skip_group_check=True,
                        )
                    # recip on the FULL tile (custom-DVE op needs base-0 full
                    # APs; rows 0:64 are garbage 1/AO values, never read).
                    rec = recp.tile([128, 512], F32, tag="rec")
                    nc.vector.reciprocal_approx_fast(out=rec[:], in_=ao[:])
                    nc.vector.tensor_tensor(
                        out=ao2T[r0:r0 + 64, sq, g * 512:(g + 1) * 512],
                        in0=ao[0:64, :], in1=rec[64:128, :],
                        op=mybir.AluOpType.mult,
                    )

            def emit_oproj(tb):
                for fc in range(8):
                    po = mmp.tile([128, 512], F32, tag="mm512")
                    for c2 in range(2):
                        nc.tensor.matmul(
                            po[:],
                            lhsT=wo_sb[:, c2, fc * 128:(fc + 1) * 128],
                            rhs=ao2T[:, c2, tb * 512:(tb + 1) * 512],
                            start=(c2 == 0), stop=(c2 == 1),
                        )
                    ev = evp.tile([128, 512], BF16, tag="ev")
                    if fc % 2 == 0:
                        nc.scalar.copy(ev[:], po[:])
                    else:
                        nc.vector.tensor_copy(ev[:], po[:])
                    nc.sync.dma_start(outT[fc, :, tb * 512:(tb + 1) * 512], ev[:])

            # --- schedule ---
            emit_qk((0, 1), nc.scalar)            # K0 Q0 (ACT idle here)
            pth0a = ptp.tile([128, NT, 640], BF16, tag="pth")
            pth0b = ptp.tile([128, NT, 640], BF16, tag="pth")
            pths0 = [pth0a, pth0b]
            for jb in range(NT):                  # S0 interleaved with V
                emit_s(0, pths0, jb)
                emit_v(jb)
            emit_qk((2, 3), nc.vector)            # K1 Q1 (ACT busy with sq0 exps)
            pth1a = ptp.tile([128, NT, 640], BF16, tag="pth")
            pth1b = ptp.tile([128, NT, 640], BF16, tag="pth")
            pths1 = [pth1a, pth1b]
            for jb in range(NT):                  # S1
                emit_s(1, pths1, jb)
            for g in range(4):                    # AV0
                emit_av_group(0, pths0, g)
            if dbg:
                nc.sync.dma_start(d_qkvT[:], qkvT[:])
                nc.sync.dma_start(d_vnat[:], vnat[:])
                nc.sync.dma_start(d_pth[:], pths0[0][:])
            for g in range(4):                    # AV1 + out-proj interleaved
                emit_av_group(1, pths1, g)
                emit_oproj(g)
            if dbg:
                nc.sync.dma_start(d_ao2T[:], ao2T[:])

    nc.compile()
    return nc


def _host_inputs(query, in_proj_weight, out_proj_weight):
    """Per-core input maps (numpy only)."""
    slopes = np.asarray(_get_slopes(H), np.float32)
    q32 = np.asarray(query, np.float32)
    w_in = np.asarray(in_proj_weight, np.float32)
    w_out = np.asarray(out_proj_weight, np.float32)

    jj = np.arange(128)[:, None]
    cc = np.arange(640)[None, :]
    allowed = (cc >= jj) & (cc - jj <= W)
    in_maps = []
    for c in range(NCORES):
        b, hq = divmod(c, 4)
        heads = np.arange(4 * hq, 4 * hq + HL)
        rows = (heads[:, None] * HD + np.arange(HD)[None, :]).reshape(-1)  # 256

        # slots K0 Q0 K1 Q1: slot s covers head pair s//2 (128 rows)
        blocks = []
        for s in range(4):
            sq, is_q = s // 2, (s % 2 == 1)
            rws = rows[sq * 128:(sq + 1) * 128]
            wb = (w_in[rws, :] * (1.0 / math.sqrt(HD))) if is_q else w_in[E + rws, :]
            blocks.append(wb)
        w_qk = np.concatenate(blocks, axis=0)                    # [512, E]
        wqk = np.ascontiguousarray(
            w_qk.T.reshape(8, 128, 512)).astype(ml_dtypes.bfloat16)

        wv = w_in[2 * E + rows, :]                               # [256, E]
        wvT_ = np.ascontiguousarray(
            wv.T.reshape(8, 128, 256)).astype(ml_dtypes.bfloat16)

        xTa = np.ascontiguousarray(
            q32[b].T.reshape(8, 128, T)).astype(ml_dtypes.bfloat16)

        wo_loc = np.ascontiguousarray(
            w_out[:, rows].T.reshape(2, 128, E)).astype(ml_dtypes.bfloat16)

        biasd = np.empty((HL, 128, 640), ml_dtypes.bfloat16)
        for hl in range(HL):
            s = slopes[4 * hq + hl]
            eb = np.where(allowed, np.exp(-s * (cc - jj).astype(np.float64)), 0.0)
            biasd[hl] = eb.astype(ml_dtypes.bfloat16)

        in_maps.append(
            {"xT": xTa, "wqk": wqk, "wvT": wvT_, "wo": wo_loc, "biasd": biasd})
    return in_maps


def _assemble(results):
    out = np.zeros((B, T, E), np.float32)
    for c in range(NCORES):
        b = c // 4
        part = np.asarray(results[c]["outT"]).astype(np.float32)  # [8,128,T]
        out[b] += part.reshape(E, T).T
    return out


def kernel(query, in_proj_weight, out_proj_weight, num_heads, sliding_window_size):
    assert int(num_heads) == H and int(sliding_window_size) == W
    assert query.shape == (B, T, E)
    if "nc" not in _CACHE:
        _CACHE["nc"] = _build()
    in_maps = _host_inputs(query, in_proj_weight, out_proj_weight)
    res = run_bass_kernel_spmd(_CACHE["nc"], in_maps, list(range(NCORES))).results
    return _assemble(res)


# revision 21
# speedup vs baseline: 1.4813x; 1.0795x over previous
"""Trainium2 Bass kernel for CachedMultiheadAttention (sliding-window + ALiBi).

Sharding: 8 cores = 2 batches x 4 head-quartets. Core c handles batch c//4 and
heads [4*(c%4), 4*(c%4)+4). Each core computes QKV projection for its heads,
banded attention (causal + 512 window + ALiBi), and a partial out-projection
over its heads' 256 embedding columns. Host sums the 4 partials per batch.

v2 design (all bf16 matmuls, PE kept saturated):
  - Q^T/K^T produced slot-by-slot so S-matmuls start 2/5 into the projection.
  - V produced in NATURAL layout straight from the projection (no PE
    transposes), with a 64-wide ones block per head: the AV matmul output
    rows 0-63 are AO^T and rows 64-127 are the softmax denominator Z
    replicated 64x -- the partition broadcast comes free from the matmul.
  - Normalize = reciprocal_approx_fast + one multiply (no serial
    reciprocal/partition_broadcast chain).
  - exp on ACT; band+ALiBi multiplicative bias on DVE/GpSimd alternating.
  - out-proj per 512-col group interleaved with the last AV groups.
"""
import math

import numpy as np
import ml_dtypes

import concourse.bass as bass
import concourse.tile as tile
from concourse import bacc, mybir
from concourse.bass_utils import run_bass_kernel_spmd

F32 = mybir.dt.float32
BF16 = mybir.dt.bfloat16

B, T, E, H, HD, W = 2, 2048, 1024, 16, 64, 512
NCORES = 8
HL = 4                # local heads per core
NT = T // 128         # 16 t-blocks

_CACHE = {}


def _get_slopes(n):
    def p2(m):
        start = 2 ** (-(2 ** (-(math.log2(m) - 3))))
        return [start * start**i for i in range(m)]
    if math.log2(n) % 1 == 0:
        return p2(n)
    c = 2 ** math.floor(math.log2(n))
    return p2(c) + _get_slopes(2 * c)[0::2][: n - c]


def _build(dbg=False):
    nc = bacc.Bacc("TRN2", target_bir_lowering=False, debug=False, num_devices=NCORES)
    xT = nc.dram_tensor("xT", [8, 128, T], BF16, kind="ExternalInput").ap()
    wqk = nc.dram_tensor("wqk", [8, 128, 512], BF16, kind="ExternalInput").ap()
    wvT = nc.dram_tensor("wvT", [8, 128, 256], BF16, kind="ExternalInput").ap()
    wo = nc.dram_tensor("wo", [2, 128, E], BF16, kind="ExternalInput").ap()
    biasd = nc.dram_tensor("biasd", [HL, 128, 640], BF16, kind="ExternalInput").ap()
    outT = nc.dram_tensor("outT", [8, 128, T], BF16, kind="ExternalOutput").ap()
    if dbg:
        d_qkvT = nc.dram_tensor("d_qkvT", [128, 4, T], BF16, kind="ExternalOutput").ap()
        d_vnat = nc.dram_tensor("d_vnat", [128, NT, HL, 128], BF16, kind="ExternalOutput").ap()
        d_pth = nc.dram_tensor("d_pth", [128, NT, 640], BF16, kind="ExternalOutput").ap()
        d_ao2T = nc.dram_tensor("d_ao2T", [128, 2, T], BF16, kind="ExternalOutput").ap()

    with tile.TileContext(nc) as tc:
        with (
            tc.tile_pool(name="singles", bufs=1) as singles,
            tc.tile_pool(name="ptp", bufs=4) as ptp,
            tc.tile_pool(name="sprep", bufs=3) as sprep,
            tc.tile_pool(name="recp", bufs=2) as recp,
            tc.tile_pool(name="evp", bufs=3) as evp,
            tc.tile_pool(name="mm", bufs=3, space="PSUM") as mmp,
            tc.tile_pool(name="s1", bufs=2, space="PSUM") as s1p,
            tc.tile_pool(name="accp", bufs=3, space="PSUM") as accp,
        ):
            # --- one-time loads (DMA queues: sync + gpsimd for x; scalar
            # carries only 3 late-needed weight loads so its queue is free
            # for the K0Q0 PSUM->SBUF copies from ~2us on) ---
            wqk_sb = singles.tile([128, 8, 512], BF16)
            for ec in range(8):
                nc.gpsimd.dma_start(
                    wqk_sb[:, ec, :], wqk.rearrange("c p m -> p c m")[:, ec, :])
            wvT_sb = singles.tile([128, 8, 256], BF16)
            nc.scalar.dma_start(wvT_sb[:], wvT.rearrange("c p m -> p c m"))
            wo_sb = singles.tile([128, 2, E], BF16)
            nc.scalar.dma_start(wo_sb[:], wo.rearrange("c p f -> p c f"))
            bias_sb = singles.tile([128, HL, 640], BF16)
            nc.scalar.dma_start(bias_sb[:], biasd.rearrange("h p c -> p h c"))

            xsb = singles.tile([128, 8, T], BF16)
            for tq in range(4):
                for ec in range(8):
                    # early chunks on sync (empty queue); late ones on gpsimd
                    eng = nc.sync if (tq * 8 + ec) < 20 else nc.gpsimd
                    eng.dma_start(
                        xsb[:, ec, tq * 512:(tq + 1) * 512],
                        xT[ec, :, tq * 512:(tq + 1) * 512])

            qkvT = singles.tile([128, 4, T], BF16)   # slots: K0 Q0 K1 Q1
            vnat = singles.tile([128, NT, HL, 128], BF16)
            nc.vector.memset(vnat[:, :, :, HD:128], 1.0)  # ones blocks only
            ao2T = singles.tile([128, 2, T], BF16)   # normalized AO^T

            # --- emit helpers ---
            def emit_qk_group(s, tb, copy_eng):
                pt = mmp.tile([128, 512], F32, tag="mm512")
                for ec in range(8):
                    nc.tensor.matmul(
                        pt[:],
                        lhsT=wqk_sb[:, ec, s * 128:(s + 1) * 128],
                        rhs=xsb[:, ec, tb * 512:(tb + 1) * 512],
                        start=(ec == 0), stop=(ec == 7),
                    )
                if copy_eng is nc.scalar:
                    nc.scalar.copy(qkvT[:, s, tb * 512:(tb + 1) * 512], pt[:])
                else:
                    copy_eng.tensor_copy(
                        qkvT[:, s, tb * 512:(tb + 1) * 512], pt[:])

            def emit_v(tb16):
                pv = mmp.tile([128, 256], F32, tag="mm512")
                for ec in range(8):
                    nc.tensor.matmul(
                        pv[:],
                        lhsT=xsb[:, ec, tb16 * 128:(tb16 + 1) * 128],
                        rhs=wvT_sb[:, ec, :],
                        start=(ec == 0), stop=(ec == 7),
                    )
                nc.vector.tensor_copy(
                    vnat[:, tb16, :, 0:HD],
                    pv[:].rearrange("p (h d) -> p h d", h=HL))

            def emit_s(sq, pths, jb):
                ks, qs = 2 * sq, 2 * sq + 1
                qw = min(5, NT - jb) * 128
                w0 = min(qw, 512)
                for hh in range(2):
                    h = 2 * sq + hh
                    r0 = hh * 64
                    pth = pths[hh]
                    praw = sprep.tile([128, 640], BF16, tag="praw")
                    s5 = mmp.tile([128, 512], F32, tag="mm512")
                    nc.tensor.matmul(
                        s5[:, 0:w0],
                        lhsT=qkvT[r0:r0 + 64, ks, jb * 128:(jb + 1) * 128],
                        rhs=qkvT[r0:r0 + 64, qs, jb * 128:jb * 128 + w0],
                        start=True, stop=True,
                    )
                    nc.scalar.activation(
                        out=praw[:, 0:w0], in_=s5[:, 0:w0],
                        func=mybir.ActivationFunctionType.Exp,
                    )
                    if qw > 512:
                        s1 = s1p.tile([128, 128], F32, tag="s128")
                        nc.tensor.matmul(
                            s1[:],
                            lhsT=qkvT[r0:r0 + 64, ks, jb * 128:(jb + 1) * 128],
                            rhs=qkvT[r0:r0 + 64, qs, jb * 128 + 512:jb * 128 + qw],
                            start=True, stop=True,
                        )
                        nc.scalar.activation(
                            out=praw[:, 512:qw], in_=s1[:],
                            func=mybir.ActivationFunctionType.Exp,
                        )
                    # sq0 phase: vector has slack -> all vector. sq1 phase:
                    # vector carries the AV recip+normalize, so gpsimd takes
                    # most bias mults despite being ~1.8x slower.
                    if sq == 0:
                        eng = nc.vector
                    else:
                        eng = nc.vector if (jb % 4 == 1 and hh == 0) else nc.gpsimd
                    eng.tensor_tensor(
                        out=pth[:, jb, 0:qw], in0=praw[:, 0:qw],
                        in1=bias_sb[:, h, 0:qw], op=mybir.AluOpType.mult,
                    )

            def emit_av_group(sq, pths, g):
                for hh in range(2):
                    h = 2 * sq + hh
                    r0 = hh * 64
                    pth = pths[hh]
                    ao = accp.tile([128, 512], F32, tag="acc")
                    jbs = [4 * g] + [jb for jb in range(max(0, 4 * g - 4), 4 * g + 4)
                                     if jb != 4 * g]
                    for i, jb in enumerate(jbs):
                        qb_lo = max(4 * g, jb)
                        qb_hi = min(4 * g + 3, jb + 4)
                        wdt = (qb_hi - qb_lo + 1) * 128
                        ao_off = (qb_lo - 4 * g) * 128
                        p_off = (qb_lo - jb) * 128
                        nc.tensor.matmul(
                            ao[:, ao_off:ao_off + wdt],
                            lhsT=vnat[:, jb, h, :],
                            rhs=pth[:, jb, p_off:p_off + wdt],
                            start=(i == 0), stop=(i == len(jbs) - 1),
                            skip_group_check=True,
                        )
                    # recip on the FULL tile (custom-DVE op needs base-0 full
                    # APs; rows 0:64 are garbage 1/AO values, never read).
                    # (gpsimd cannot touch PSUM; both stages live on vector)
                    rec = recp.tile([128, 512], F32, tag="rec")
                    nc.vector.reciprocal_approx_fast(out=rec[:], in_=ao[:])
                    nc.vector.tensor_tensor(
                        out=ao2T[r0:r0 + 64, sq, g * 512:(g + 1) * 512],
                        in0=ao[0:64, :], in1=rec[64:128, :],
                        op=mybir.AluOpType.mult,
                    )

            def emit_oproj(tb):
                for fc in range(8):
                    po = mmp.tile([128, 512], F32, tag="mm512")
                    for c2 in range(2):
                        nc.tensor.matmul(
                            po[:],
                            lhsT=wo_sb[:, c2, fc * 128:(fc + 1) * 128],
                            rhs=ao2T[:, c2, tb * 512:(tb + 1) * 512],
                            start=(c2 == 0), stop=(c2 == 1),
                        )
                    ev = evp.tile([128, 512], BF16, tag="ev")
                    if fc % 2 == 0:
                        nc.scalar.copy(ev[:], po[:])
                    else:
                        nc.vector.tensor_copy(ev[:], po[:])
                    nc.sync.dma_start(outT[fc, :, tb * 512:(tb + 1) * 512], ev[:])

            # --- schedule (PE queue kept dense; ACT-paced stretches get
            # independent PE work folded in) ---
            for tb in range(4):                   # K0 Q0, tb-major (DMA-paced)
                for s in (0, 1):
                    emit_qk_group(s, tb, nc.scalar)
            pth0a = ptp.tile([128, NT, 640], BF16, tag="pth")
            pth0b = ptp.tile([128, NT, 640], BF16, tag="pth")
            pths0 = [pth0a, pth0b]
            k1q1 = [(s, tb) for tb in range(4) for s in (2, 3)]
            for jb in range(NT):                  # S0 + V + K1Q1 interleaved
                emit_s(0, pths0, jb)
                emit_v(jb)
                if jb % 2 == 0:
                    s, tb = k1q1[jb // 2]
                    emit_qk_group(s, tb, nc.vector)
            pth1a = ptp.tile([128, NT, 640], BF16, tag="pth")
            pth1b = ptp.tile([128, NT, 640], BF16, tag="pth")
            pths1 = [pth1a, pth1b]
            for b in range(4):                    # rolling S1 / AV0 / AV1 / oproj
                for jb in range(4 * b, 4 * b + 4):
                    emit_s(1, pths1, jb)
                emit_av_group(0, pths0, b)
                if b >= 1:
                    emit_av_group(1, pths1, b - 1)
                    emit_oproj(b - 1)
            if dbg:
                nc.sync.dma_start(d_qkvT[:], qkvT[:])
                nc.sync.dma_start(d_vnat[:], vnat[:])
                nc.sync.dma_start(d_pth[:], pths0[0][:])
            emit_av_group(1, pths1, 3)
            emit_oproj(3)
            if dbg:
                nc.sync.dma_start(d_ao2T[:], ao2T[:])

    nc.compile()
    return nc


def _host_inputs(query, in_proj_weight, out_proj_weight):
    """Per-core input maps (numpy only)."""
    slopes = np.asarray(_get_slopes(H), np.float32)
    q32 = np.asarray(query, np.float32)
    w_in = np.asarray(in_proj_weight, np.float32)
    w_out = np.asarray(out_proj_weight, np.float32)

    jj = np.arange(128)[:, None]
    cc = np.arange(640)[None, :]
    allowed = (cc >= jj) & (cc - jj <= W)
    in_maps = []
    for c in range(NCORES):
        b, hq = divmod(c, 4)
        heads = np.arange(4 * hq, 4 * hq + HL)
        rows = (heads[:, None] * HD + np.arange(HD)[None, :]).reshape(-1)  # 256

        # slots K0 Q0 K1 Q1: slot s covers head pair s//2 (128 rows)
        blocks = []
        for s in range(4):
            sq, is_q = s // 2, (s % 2 == 1)
            rws = rows[sq * 128:(sq + 1) * 128]
            wb = (w_in[rws, :] * (1.0 / math.sqrt(HD))) if is_q else w_in[E + rws, :]
            blocks.append(wb)
        w_qk = np.concatenate(blocks, axis=0)                    # [512, E]
        wqk = np.ascontiguousarray(
            w_qk.T.reshape(8, 128, 512)).astype(ml_dtypes.bfloat16)

        wv = w_in[2 * E + rows, :]                               # [256, E]
        wvT_ = np.ascontiguousarray(
            wv.T.reshape(8, 128, 256)).astype(ml_dtypes.bfloat16)

        xTa = np.ascontiguousarray(
            q32[b].T.reshape(8, 128, T)).astype(ml_dtypes.bfloat16)

        wo_loc = np.ascontiguousarray(
            w_out[:, rows].T.reshape(2, 128, E)).astype(ml_dtypes.bfloat16)

        biasd = np.empty((HL, 128, 640), ml_dtypes.bfloat16)
        for hl in range(HL):
            s = slopes[4 * hq + hl]
            eb = np.where(allowed, np.exp(-s * (cc - jj).astype(np.float64)), 0.0)
            biasd[hl] = eb.astype(ml_dtypes.bfloat16)

        in_maps.append(
            {"xT": xTa, "wqk": wqk, "wvT": wvT_, "wo": wo_loc, "biasd": biasd})
    return in_maps


def _assemble(results):
    out = np.zeros((B, T, E), np.float32)
    for c in range(NCORES):
        b = c // 4
        part = np.asarray(results[c]["outT"]).astype(np.float32)  # [8,128,T]
        out[b] += part.reshape(E, T).T
    return out


def kernel(query, in_proj_weight, out_proj_weight, num_heads, sliding_window_size):
    assert int(num_heads) == H and int(sliding_window_size) == W
    assert query.shape == (B, T, E)
    if "nc" not in _CACHE:
        _CACHE["nc"] = _build()
    in_maps = _host_inputs(query, in_proj_weight, out_proj_weight)
    res = run_bass_kernel_spmd(_CACHE["nc"], in_maps, list(range(NCORES))).results
    return _assemble(res)


# revision 30
# speedup vs baseline: 1.6099x; 1.0868x over previous
"""Trainium2 Bass kernel for CachedMultiheadAttention (sliding-window + ALiBi).

Sharding: 8 cores = 2 batches x 4 head-quartets. Core c handles batch c//4 and
heads [4*(c%4), 4*(c%4)+4). Each core computes QKV projection for its heads,
banded attention (causal + 512 window + ALiBi), and a partial out-projection
over its heads' 256 embedding columns. Host sums the 4 partials per batch.

v2 design (all bf16 matmuls, PE kept saturated):
  - Q^T/K^T produced slot-by-slot so S-matmuls start 2/5 into the projection.
  - V produced in NATURAL layout straight from the projection (no PE
    transposes), with a 64-wide ones block per head: the AV matmul output
    rows 0-63 are AO^T and rows 64-127 are the softmax denominator Z
    replicated 64x -- the partition broadcast comes free from the matmul.
  - Normalize = reciprocal_approx_fast + one multiply (no serial
    reciprocal/partition_broadcast chain).
  - exp on ACT; band+ALiBi multiplicative bias on DVE/GpSimd alternating.
  - out-proj per 512-col group interleaved with the last AV groups.
"""
import math

import numpy as np
import ml_dtypes

import concourse.bass as bass
import concourse.tile as tile
from concourse import bacc, mybir
from concourse.bass_utils import run_bass_kernel_spmd

F32 = mybir.dt.float32
BF16 = mybir.dt.bfloat16

B, T, E, H, HD, W = 2, 2048, 1024, 16, 64, 512
NCORES = 8
HL = 4                # local heads per core
NT = T // 128         # 16 t-blocks

_CACHE = {}


def _get_slopes(n):
    def p2(m):
        start = 2 ** (-(2 ** (-(math.log2(m) - 3))))
        return [start * start**i for i in range(m)]
    if math.log2(n) % 1 == 0:
        return p2(n)
    c = 2 ** math.floor(math.log2(n))
    return p2(c) + _get_slopes(2 * c)[0::2][: n - c]


def _build(dbg=False):
    nc = bacc.Bacc("TRN2", target_bir_lowering=False, debug=False, num_devices=NCORES)
    xT = nc.dram_tensor("xT", [8, 128, T], BF16, kind="ExternalInput").ap()
    wqk = nc.dram_tensor("wqk", [8, 128, 512], BF16, kind="ExternalInput").ap()
    wvT = nc.dram_tensor("wvT", [8, 128, 256], BF16, kind="ExternalInput").ap()
    wo = nc.dram_tensor("wo", [2, 128, E], BF16, kind="ExternalInput").ap()
    biasd = nc.dram_tensor("biasd", [HL, 128, 640], BF16, kind="ExternalInput").ap()
    outT = nc.dram_tensor("outT", [8, 128, T], BF16, kind="ExternalOutput").ap()
    if dbg:
        d_qkvT = nc.dram_tensor("d_qkvT", [128, 4, T], BF16, kind="ExternalOutput").ap()
        d_vnat = nc.dram_tensor("d_vnat", [128, NT, HL, 128], BF16, kind="ExternalOutput").ap()
        d_pth = nc.dram_tensor("d_pth", [128, NT, 640], BF16, kind="ExternalOutput").ap()
        d_ao2T = nc.dram_tensor("d_ao2T", [128, 2, T], BF16, kind="ExternalOutput").ap()

    with tile.TileContext(nc) as tc:
        with (
            tc.tile_pool(name="singles", bufs=1) as singles,
            tc.tile_pool(name="ptp", bufs=4) as ptp,
            tc.tile_pool(name="sprep", bufs=3) as sprep,
            tc.tile_pool(name="recp", bufs=2) as recp,
            tc.tile_pool(name="evp", bufs=3) as evp,
            tc.tile_pool(name="mm", bufs=3, space="PSUM") as mmp,
            tc.tile_pool(name="s1", bufs=2, space="PSUM") as s1p,
            tc.tile_pool(name="accp", bufs=3, space="PSUM") as accp,
        ):
            # --- one-time loads: few big descriptors (the ~650ns per-trigger
            # queue cost dominates small chunks). wqk single-shot gates the
            # first matmul; x split [ec-half x tb] across sync/scalar. ---
            wqk_sb = singles.tile([128, 8, 512], BF16)
            nc.gpsimd.dma_start(wqk_sb[:], wqk.rearrange("c p m -> p c m"))
            xsb = singles.tile([128, 8, T], BF16)
            xTr = xT.rearrange("c p t -> p c t")
            for tb in range(4):
                nc.sync.dma_start(
                    xsb[:, 0:4, tb * 512:(tb + 1) * 512],
                    xTr[:, 0:4, tb * 512:(tb + 1) * 512])
                nc.scalar.dma_start(
                    xsb[:, 4:8, tb * 512:(tb + 1) * 512],
                    xTr[:, 4:8, tb * 512:(tb + 1) * 512])
            wvT_sb = singles.tile([128, 8, 256], BF16)
            nc.gpsimd.dma_start(wvT_sb[:], wvT.rearrange("c p m -> p c m"))
            bias_sb = singles.tile([128, HL, 640], BF16)
            nc.gpsimd.dma_start(bias_sb[:], biasd.rearrange("h p c -> p h c"))
            wo_sb = singles.tile([128, 2, E], BF16)
            nc.gpsimd.dma_start(wo_sb[:], wo.rearrange("c p f -> p c f"))

            qkvT = singles.tile([128, 4, T], BF16)   # slots: K0 Q0 K1 Q1
            vnat = singles.tile([128, NT, HL, 128], BF16)
            nc.vector.memset(vnat[:, :, :, HD:128], 1.0)  # ones blocks only
            ao2T = singles.tile([128, 2, T], BF16)   # normalized AO^T

            # --- emit helpers ---
            def emit_qk_group(s, tb, copy_eng):
                pt = mmp.tile([128, 512], F32, tag="mm512")
                for ec in range(8):
                    nc.tensor.matmul(
                        pt[:],
                        lhsT=wqk_sb[:, ec, s * 128:(s + 1) * 128],
                        rhs=xsb[:, ec, tb * 512:(tb + 1) * 512],
                        start=(ec == 0), stop=(ec == 7),
                    )
                if copy_eng is nc.scalar:
                    nc.scalar.copy(qkvT[:, s, tb * 512:(tb + 1) * 512], pt[:])
                else:
                    copy_eng.tensor_copy(
                        qkvT[:, s, tb * 512:(tb + 1) * 512], pt[:])

            def emit_v(tb16):
                pv = mmp.tile([128, 256], F32, tag="mm512")
                for ec in range(8):
                    nc.tensor.matmul(
                        pv[:],
                        lhsT=xsb[:, ec, tb16 * 128:(tb16 + 1) * 128],
                        rhs=wvT_sb[:, ec, :],
                        start=(ec == 0), stop=(ec == 7),
                    )
                nc.scalar.copy(
                    vnat[:, tb16, :, 0:HD],
                    pv[:].rearrange("p (h d) -> p h d", h=HL))

            def emit_s(sq, pths, jb):
                ks, qs = 2 * sq, 2 * sq + 1
                qw = min(5, NT - jb) * 128
                w0 = min(qw, 512)
                for hh in range(2):
                    h = 2 * sq + hh
                    r0 = hh * 64
                    pth = pths[hh]
                    praw = sprep.tile([128, 640], BF16, tag="praw")
                    s5 = mmp.tile([128, 512], F32, tag="mm512")
                    nc.tensor.matmul(
                        s5[:, 0:w0],
                        lhsT=qkvT[r0:r0 + 64, ks, jb * 128:(jb + 1) * 128],
                        rhs=qkvT[r0:r0 + 64, qs, jb * 128:jb * 128 + w0],
                        start=True, stop=True,
                    )
                    nc.scalar.activation(
                        out=praw[:, 0:w0], in_=s5[:, 0:w0],
                        func=mybir.ActivationFunctionType.Exp,
                    )
                    if qw > 512:
                        s1 = s1p.tile([128, 128], F32, tag="s128")
                        nc.tensor.matmul(
                            s1[:],
                            lhsT=qkvT[r0:r0 + 64, ks, jb * 128:(jb + 1) * 128],
                            rhs=qkvT[r0:r0 + 64, qs, jb * 128 + 512:jb * 128 + qw],
                            start=True, stop=True,
                        )
                        nc.scalar.activation(
                            out=praw[:, 512:qw], in_=s1[:],
                            func=mybir.ActivationFunctionType.Exp,
                        )
                    # vector carries the PSUM-only AV recip/normalize chains,
                    # so bias mults spill to gpsimd (~1.8x slower) where
                    # vector is loaded: late phase-A and the rounds.
                    if sq == 0:
                        eng = nc.vector if (hh == 0 or jb < 8) else nc.gpsimd
                    else:
                        eng = nc.vector if (hh == 0 and jb % 2 == 0) else nc.gpsimd
                    eng.tensor_tensor(
                        out=pth[:, jb, 0:qw], in0=praw[:, 0:qw],
                        in1=bias_sb[:, h, 0:qw], op=mybir.AluOpType.mult,
                    )

            def emit_av_group(sq, pths, g):
                for hh in range(2):
                    h = 2 * sq + hh
                    r0 = hh * 64
                    pth = pths[hh]
                    ao = accp.tile([128, 512], F32, tag="acc")
                    jbs = [4 * g] + [jb for jb in range(max(0, 4 * g - 4), 4 * g + 4)
                                     if jb != 4 * g]
                    for i, jb in enumerate(jbs):
                        qb_lo = max(4 * g, jb)
                        qb_hi = min(4 * g + 3, jb + 4)
                        wdt = (qb_hi - qb_lo + 1) * 128
                        ao_off = (qb_lo - 4 * g) * 128
                        p_off = (qb_lo - jb) * 128
                        nc.tensor.matmul(
                            ao[:, ao_off:ao_off + wdt],
                            lhsT=vnat[:, jb, h, :],
                            rhs=pth[:, jb, p_off:p_off + wdt],
                            start=(i == 0), stop=(i == len(jbs) - 1),
                            skip_group_check=True,
                        )
                    # recip on the FULL tile (custom-DVE op needs base-0 full
                    # APs; rows 0:64 are garbage 1/AO values, never read).
                    # (gpsimd cannot touch PSUM; both stages live on vector)
                    rec = recp.tile([128, 512], F32, tag="rec")
                    nc.vector.reciprocal_approx_fast(out=rec[:], in_=ao[:])
                    nc.vector.tensor_tensor(
                        out=ao2T[r0:r0 + 64, sq, g * 512:(g + 1) * 512],
                        in0=ao[0:64, :], in1=rec[64:128, :],
                        op=mybir.AluOpType.mult,
                    )

            def emit_oproj(tb):
                for fc in range(8):
                    po = mmp.tile([128, 512], F32, tag="mm512")
                    for c2 in range(2):
                        nc.tensor.matmul(
                            po[:],
                            lhsT=wo_sb[:, c2, fc * 128:(fc + 1) * 128],
                            rhs=ao2T[:, c2, tb * 512:(tb + 1) * 512],
                            start=(c2 == 0), stop=(c2 == 1),
                        )
                    ev = evp.tile([128, 512], BF16, tag="ev")
                    if fc % 2 == 0:
                        nc.scalar.copy(ev[:], po[:])
                    else:
                        nc.vector.tensor_copy(ev[:], po[:])
                    nc.sync.dma_start(outT[fc, :, tb * 512:(tb + 1) * 512], ev[:])

            # --- schedule (PE queue kept dense; ACT-paced stretches get
            # independent PE work folded in) ---
            for tb in range(4):                   # K0 Q0, tb-major (DMA-paced)
                for s in (0, 1):
                    emit_qk_group(s, tb, nc.scalar)
            pth0a = ptp.tile([128, NT, 640], BF16, tag="pth")
            pth0b = ptp.tile([128, NT, 640], BF16, tag="pth")
            pths0 = [pth0a, pth0b]
            k1q1 = [(s, tb) for tb in range(4) for s in (2, 3)]
            for jb in range(NT):                  # S0 + V + K1Q1 interleaved
                emit_s(0, pths0, jb)
                emit_v(jb)
                if jb % 2 == 0:
                    s, tb = k1q1[jb // 2]
                    emit_qk_group(s, tb, nc.vector)
            pth1a = ptp.tile([128, NT, 640], BF16, tag="pth")
            pth1b = ptp.tile([128, NT, 640], BF16, tag="pth")
            pths1 = [pth1a, pth1b]
            for b in range(4):                    # rolling S1 / AV0 / AV1 / oproj
                for jb in range(4 * b, 4 * b + 4):
                    emit_s(1, pths1, jb)
                emit_av_group(0, pths0, b)
                if b >= 1:
                    emit_av_group(1, pths1, b - 1)
                    emit_oproj(b - 1)
            if dbg:
                nc.sync.dma_start(d_qkvT[:], qkvT[:])
                nc.sync.dma_start(d_vnat[:], vnat[:])
                nc.sync.dma_start(d_pth[:], pths0[0][:])
            emit_av_group(1, pths1, 3)
            emit_oproj(3)
            if dbg:
                nc.sync.dma_start(d_ao2T[:], ao2T[:])

    nc.compile()
    return nc


def _host_inputs(query, in_proj_weight, out_proj_weight):
    """Per-core input maps (numpy only)."""
    slopes = np.asarray(_get_slopes(H), np.float32)
    q32 = np.asarray(query, np.float32)
    w_in = np.asarray(in_proj_weight, np.float32)
    w_out = np.asarray(out_proj_weight, np.float32)

    jj = np.arange(128)[:, None]
    cc = np.arange(640)[None, :]
    allowed = (cc >= jj) & (cc - jj <= W)
    in_maps = []
    for c in range(NCORES):
        b, hq = divmod(c, 4)
        heads = np.arange(4 * hq, 4 * hq + HL)
        rows = (heads[:, None] * HD + np.arange(HD)[None, :]).reshape(-1)  # 256

        # slots K0 Q0 K1 Q1: slot s covers head pair s//2 (128 rows)
        blocks = []
        for s in range(4):
            sq, is_q = s // 2, (s % 2 == 1)
            rws = rows[sq * 128:(sq + 1) * 128]
            wb = (w_in[rws, :] * (1.0 / math.sqrt(HD))) if is_q else w_in[E + rws, :]
            blocks.append(wb)
        w_qk = np.concatenate(blocks, axis=0)                    # [512, E]
        wqk = np.ascontiguousarray(
            w_qk.T.reshape(8, 128, 512)).astype(ml_dtypes.bfloat16)

        wv = w_in[2 * E + rows, :]                               # [256, E]
        wvT_ = np.ascontiguousarray(
            wv.T.reshape(8, 128, 256)).astype(ml_dtypes.bfloat16)

        xTa = np.ascontiguousarray(
            q32[b].T.reshape(8, 128, T)).astype(ml_dtypes.bfloat16)

        wo_loc = np.ascontiguousarray(
            w_out[:, rows].T.reshape(2, 128, E)).astype(ml_dtypes.bfloat16)

        biasd = np.empty((HL, 128, 640), ml_dtypes.bfloat16)
        for hl in range(HL):
            s = slopes[4 * hq + hl]
            eb = np.where(allowed, np.exp(-s * (cc - jj).astype(np.float64)), 0.0)
            biasd[hl] = eb.astype(ml_dtypes.bfloat16)

        in_maps.append(
            {"xT": xTa, "wqk": wqk, "wvT": wvT_, "wo": wo_loc, "biasd": biasd})
    return in_maps


def _assemble(results):
    out = np.zeros((B, T, E), np.float32)
    for c in range(NCORES):
        b = c // 4
        part = np.asarray(results[c]["outT"]).astype(np.float32)  # [8,128,T]
        out[b] += part.reshape(E, T).T
    return out


def kernel(query, in_proj_weight, out_proj_weight, num_heads, sliding_window_size):
    assert int(num_heads) == H and int(sliding_window_size) == W
    assert query.shape == (B, T, E)
    if "nc" not in _CACHE:
        _CACHE["nc"] = _build()
    in_maps = _host_inputs(query, in_proj_weight, out_proj_weight)
    res = run_bass_kernel_spmd(_CACHE["nc"], in_maps, list(range(NCORES))).results
    return _assemble(res)
